# revision 1
# baseline (speedup 1.0000x reference)
"""Multi-head attention (B=4, S=2048, D=512, H=8) on 8 Trainium2 NeuronCores.

Sharding: core c handles batch b = c//2 and query-half h = c%2 (1024 queries).
Each core computes q = (x_q @ Wq.T + bq)/sqrt(hd) for its queries, k/v
projections for its batch's full 2048 keys, full softmax attention for all 8
heads, and the output projection for its query rows.  Output rows across
cores are disjoint, so there are no collectives.

On-chip layout is feature-major ("transposed activations"): scores are built
directly as S^T[k, q] so the attn @ V contraction needs no transposes, and
exp(S^T) row-sums come free via a ones-column appended to V.
"""

import numpy as np
import ml_dtypes

B = 4
S = 2048
D = 512
H = 8
HD = 64
SQ = 1024  # queries per core
N_CORES = 8
F32 = None  # set lazily (mybir dtypes) in _build
BF16 = None

_cache = {}


def _build():
    """Build (once) the SPMD Bass program shared by all 8 cores."""
    import concourse.bacc as bacc
    import concourse.mybir as mybir
    import concourse.tile as tile

    f32 = mybir.dt.float32
    bf16 = mybir.dt.bfloat16
    AF = mybir.ActivationFunctionType
    OP = mybir.AluOpType

    nc = bacc.Bacc("TRN2", target_bir_lowering=False, debug=False)

    # Per-core inputs (pre-transposed / pre-cast on host).
    xqT = nc.dram_tensor("xqT", [D, SQ], bf16, kind="ExternalInput").ap()
    keyT = nc.dram_tensor("keyT", [D, S], bf16, kind="ExternalInput").ap()
    valT = nc.dram_tensor("valT", [D, S], bf16, kind="ExternalInput").ap()
    wqT = nc.dram_tensor("wqT", [D, D], bf16, kind="ExternalInput").ap()
    wkT = nc.dram_tensor("wkT", [D, D], bf16, kind="ExternalInput").ap()
    wvT = nc.dram_tensor("wvT", [D, D], bf16, kind="ExternalInput").ap()
    woT = nc.dram_tensor("woT", [D, D], bf16, kind="ExternalInput").ap()
    bqr = nc.dram_tensor("bqr", [128, 4], f32, kind="ExternalInput").ap()
    bkr = nc.dram_tensor("bkr", [128, 4], f32, kind="ExternalInput").ap()
    bop = nc.dram_tensor("bop", [1, D], bf16, kind="ExternalInput").ap()
    ident = nc.dram_tensor("ident", [128, 128], f32, kind="ExternalInput").ap()
    y = nc.dram_tensor("y", [SQ, D], f32, kind="ExternalOutput").ap()

    with tile.TileContext(nc) as tc:
        import contextlib

        with contextlib.ExitStack() as ctx:
            const = ctx.enter_context(tc.tile_pool(name="const", bufs=1))
            io = ctx.enter_context(tc.tile_pool(name="io", bufs=1))
            acts = ctx.enter_context(tc.tile_pool(name="acts", bufs=1))
            expp = ctx.enter_context(tc.tile_pool(name="expp", bufs=20))
            rpool = ctx.enter_context(tc.tile_pool(name="rpool", bufs=2))
            dramp = ctx.enter_context(
                tc.tile_pool(name="dramp", bufs=2, space="DRAM")
            )
            psA = ctx.enter_context(tc.tile_pool(name="psA", bufs=2, space="PSUM"))
            psB = ctx.enter_context(tc.tile_pool(name="psB", bufs=4, space="PSUM"))

            # ---- constants / weights -------------------------------------
            wq_sb = const.tile([128, 4, D], bf16)
            wk_sb = const.tile([128, 4, D], bf16)
            wv_sb = const.tile([128, 4, D], bf16)
            wo_sb = const.tile([128, 4, D], bf16)
            bq_sb = const.tile([128, 4], f32)
            bk_sb = const.tile([128, 4], f32)
            bop_sb = const.tile([1, D], bf16)
            ones_row = const.tile([1, 128], bf16)
            nc.vector.memset(ones_row[:], 1.0)
            id_sb = const.tile([128, 128], f32)
            # per-head softmax denominators, feature-major: row h = sum_k e^s
            sums_sb = const.tile([8, SQ], f32)
            nc.vector.memset(sums_sb[:], 1.0)

            # ---- inputs (q/k path first: it gates the first exp) ----------
            xq_sb = io.tile([128, 4, SQ], bf16)
            key_sb = io.tile([128, 4, S], bf16)
            val_sb = io.tile([128, 4, S], bf16)
            xq_r = xqT.rearrange("(c p) s -> p c s", p=128)
            key_r = keyT.rearrange("(c p) s -> p c s", p=128)
            val_r = valT.rearrange("(c p) s -> p c s", p=128)
            wq_r = wqT.rearrange("(c p) e -> p c e", p=128)
            wk_r = wkT.rearrange("(c p) e -> p c e", p=128)
            for dc in range(4):
                nc.sync.dma_start(wq_sb[:, dc, :], wq_r[:, dc, :])
                nc.sync.dma_start(xq_sb[:, dc, :], xq_r[:, dc, :])
                nc.sync.dma_start(wk_sb[:, dc, :], wk_r[:, dc, :])
                nc.sync.dma_start(key_sb[:, dc, :], key_r[:, dc, :])
            nc.sync.dma_start(bq_sb[:], bqr[:])
            nc.sync.dma_start(bk_sb[:], bkr[:])
            nc.sync.dma_start(wv_sb[:], wvT.rearrange("(c p) e -> p c e", p=128))
            nc.sync.dma_start(wo_sb[:], woT.rearrange("(c p) e -> p c e", p=128))
            nc.sync.dma_start(bop_sb[:], bop[:])
            nc.sync.dma_start(id_sb[:], ident[:])
            for dc in range(4):
                nc.sync.dma_start(val_sb[:, dc, :], val_r[:, dc, :])

            # ---- projections ---------------------------------------------
            qT_sb = acts.tile([128, 4, SQ], bf16)  # q^T / 8, feature-major
            kT_sb = acts.tile([128, 4, S], bf16)  # k^T, feature-major
            # v natural [s, e] per k-tile, 65th column = 1.0 (row-sum trick)
            v_sb = acts.tile([128, 16, H, HD + 1], bf16)
            nc.vector.memset(v_sb[:, :, :, HD : HD + 1], 1.0)

            # q^T[e, s] = sum_d WqT[d, e] x^T[d, s]
            def emit_qproj(et):
                ps = psA.tile([128, SQ], f32, tag="psA", name=f"psq{et}")
                for dc in range(4):
                    for qn in range(2):
                        nc.tensor.matmul(
                            ps[:, qn * 512 : (qn + 1) * 512],
                            lhsT=wq_sb[:, dc, et * 128 : (et + 1) * 128],
                            rhs=xq_sb[:, dc, qn * 512 : (qn + 1) * 512],
                            start=(dc == 0),
                            stop=(dc == 3),
                        )
                nc.vector.tensor_scalar(
                    qT_sb[:, et, :], ps[:], bq_sb[:, et : et + 1], 0.125,
                    OP.add, OP.mult,
                )

            # k^T[e, s] likewise (no scale)
            def emit_kproj_half(et, kn):
                ps = psA.tile([128, SQ], f32, tag="psA", name=f"psk{et}_{kn}")
                for dc in range(4):
                    for qn in range(2):
                        o = kn * 1024 + qn * 512
                        nc.tensor.matmul(
                            ps[:, qn * 512 : (qn + 1) * 512],
                            lhsT=wk_sb[:, dc, et * 128 : (et + 1) * 128],
                            rhs=key_sb[:, dc, o : o + 512],
                            start=(dc == 0),
                            stop=(dc == 3),
                        )
                nc.vector.tensor_scalar(
                    kT_sb[:, et, kn * 1024 : (kn + 1) * 1024], ps[:],
                    bk_sb[:, et : et + 1], None, OP.add,
                )

            def emit_kproj(et):
                for kn in range(2):
                    emit_kproj_half(et, kn)

            # v[s, e] = sum_d v^T[d, s] WvT[d, e]   (bias folded into bo')
            def emit_vproj(st):
                psv = psA.tile([128, 512], f32, tag="psA", name=f"psv{st}")
                for dc in range(4):
                    nc.tensor.matmul(
                        psv[:],
                        lhsT=val_sb[:, dc, st * 128 : (st + 1) * 128],
                        rhs=wv_sb[:, dc, :],
                        start=(dc == 0),
                        stop=(dc == 3),
                    )
                nc.vector.tensor_copy(
                    v_sb[:, st, :, 0:HD],
                    psv[:].rearrange("p (h d) -> p h d", h=H),
                )

            # Emit q/k et=0 projections then pair-0 scores+exp BEFORE the
            # remaining projections, so the Scalar engine (softmax exp, the
            # co-bottleneck) starts ~15us in instead of ~60us.  Pair-0 AV is
            # emitted later (after v-proj) to keep accumulation groups from
            # blocking.
            def emit_scores_exp(hp, kt, exp_tiles):
                st_ps = [None, None]
                for hh in range(2):
                    lo = 64 * hh
                    st_ps[hh] = psA.tile(
                        [128, SQ], f32, tag="psA", name=f"st{hp}_{kt}_{hh}"
                    )
                    for qn in range(2):
                        nc.tensor.matmul(
                            st_ps[hh][:, qn * 512 : (qn + 1) * 512],
                            lhsT=kT_sb[lo : lo + 64, hp, kt * 128 : (kt + 1) * 128],
                            rhs=qT_sb[lo : lo + 64, hp, qn * 512 : (qn + 1) * 512],
                            start=True,
                            stop=True,
                            tile_position=(64 * hh, 0),
                        )
                for hh in range(2):
                    e = expp.tile([128, SQ], bf16, tag="exp",
                                  name=f"exp{hp}_{kt}_{hh}")
                    exp_tiles[hh][kt] = e
                    nc.scalar.activation(e[:], st_ps[hh][:], AF.Exp)

            emit_qproj(0)
            emit_kproj(0)
            proj_units = []
            for _et in range(1, 4):
                proj_units.append(lambda et=_et: emit_qproj(et))
                proj_units.append(lambda et=_et: emit_kproj_half(et, 0))
                proj_units.append(lambda et=_et: emit_kproj_half(et, 1))
            exp0 = [[None] * 16, [None] * 16]
            for _kt in range(16):
                emit_scores_exp(0, _kt, exp0)
                emit_vproj(_kt)
                if _kt >= 2 and _kt % 2 == 0 and proj_units:
                    proj_units.pop(0)()
            while proj_units:
                proj_units.pop(0)()

            # ---- attention (head pairs share one 128-row tile) ------------
            pending_norm = {}

            def emit_recip(hp):
                # invert the softmax denominators in a TRANSPOSED layout: a
                # [1,1024] DVE reciprocal is ~6.5us (one lane); transposing
                # via the PE makes it [128,64] (~0.5us).
                tp = psA.tile([128, 8, 8], f32, tag="psA", name=f"tp{hp}")
                for b in range(8):
                    nc.tensor.transpose(
                        tp[:, b, :],
                        sums_sb[0:8, b * 128 : (b + 1) * 128],
                        id_sb[0:8, 0:8],
                    )
                rcp = rpool.tile([128, 8, 8], f32, tag="rcp", name=f"rcp{hp}")
                nc.vector.reciprocal(rcp[:], tp[:])
                return rcp

            def emit_norm(hp):
                pair_out, avsbs = pending_norm.pop(hp)
                rcp = emit_recip(hp)
                for hh in range(2):
                    rback = psA.tile([8, 128], f32, tag="psA",
                                     name=f"rback{hp}_{hh}")
                    nc.tensor.transpose(
                        rback[:], rcp[:, :, 2 * hp + hh], id_sb[0:128, :]
                    )
                    rr8 = rpool.tile([8, 128], f32, tag="rr8",
                                     name=f"rr8{hp}_{hh}")
                    nc.vector.tensor_copy(rr8[:], rback[:])
                    scr = dramp.tile([1, SQ], f32, tag="scr",
                                     name=f"scr{hp}_{hh}")
                    nc.sync.dma_start(
                        scr[:].rearrange("x (a b) -> (x a) b", a=8), rr8[:]
                    )
                    rb = rpool.tile([64, SQ], f32, tag="rb",
                                    name=f"rb{hp}_{hh}")
                    nc.sync.dma_start(rb[:], scr[:].to_broadcast((64, SQ)))
                    nc.vector.tensor_tensor(
                        pair_out[64 * hh : 64 * hh + 64, :],
                        avsbs[hh][0:HD, :],
                        rb[:],
                        OP.mult,
                    )

            outT = []  # 4 pair tiles [128, SQ] = attn-out^T, normalized
            for hp in range(4):
                pair_out = acts.tile([128, SQ], bf16, tag=f"outT{hp}")
                outT.append(pair_out)
                av = [
                    [
                        psB.tile(
                            [HD + 1, 512], f32, tag="psB",
                            name=f"av{hp}_{hh}_{qc}",
                        )
                        for qc in range(2)
                    ]
                    for hh in range(2)
                ]
                exp_tiles = exp0 if hp == 0 else [[None] * 16, [None] * 16]
                for kt in range(16):
                    if hp != 0:
                        emit_scores_exp(hp, kt, exp_tiles)
                    if kt == 4 and (hp - 1) in pending_norm:
                        emit_norm(hp - 1)
                    for hh in range(2):
                        h = 2 * hp + hh
                        for qc in range(2):
                            nc.tensor.matmul(
                                av[hh][qc][:],
                                lhsT=v_sb[:, kt, h, :],
                                rhs=exp_tiles[hh][kt][:, qc * 512 : (qc + 1) * 512],
                                start=(kt == 0),
                                stop=(kt == 15),
                            )
                # normalize: out^T[dh, q] = av[dh, q] / av[64, q]
                # Copy PSUM -> SBUF first so the accumulators recycle fast
                # (keeps the PE fed across pair boundaries), then do the
                # recip/broadcast/multiply entirely in SBUF off the critical
                # path.
                avsbs = []
                for hh in range(2):
                    avsb = rpool.tile([HD + 1, SQ], f32, tag="avsb",
                                      name=f"avsb{hp}_{hh}")
                    avsbs.append(avsb)
                    for qc in range(2):
                        nc.vector.tensor_copy(
                            avsb[:, qc * 512 : (qc + 1) * 512], av[hh][qc][:]
                        )
                # collect the two denominator rows (gpsimd queue: sync
                # carries the input/output streams)
                for hh in range(2):
                    nc.gpsimd.dma_start(
                        sums_sb[2 * hp + hh : 2 * hp + hh + 1, :],
                        avsbs[hh][HD : HD + 1, :],
                    )
                if hp == 3:
                    # stage B consumes pair 3 unnormalized (scaled per head
                    # by the transposed reciprocal after its projection)
                    for hh in range(2):
                        nc.vector.tensor_copy(
                            pair_out[64 * hh : 64 * hh + 64, :],
                            avsbs[hh][0:HD, :],
                        )
                else:
                    pending_norm[hp] = (pair_out, avsbs)

            # ---- output projection ---------------------------------------
            # y[q, o] = sum_e outT[e, q] WoT[e, o] + bo'
            # Two stages: pairs 0-2 accumulate into SBUF as soon as the last
            # exp frees the score PSUM slots (overlaps pair-3's normalization
            # chain and keeps the PE warm); pair 3 + bias finish after its
            # normalization, off a short critical path.
            y_acc = acts.tile([128, 8, 512], f32)
            for stq in range(8):
                psy = psA.tile([128, 512], f32, tag="psA", name=f"psyA{stq}")
                for c in range(3):
                    nc.tensor.matmul(
                        psy[:],
                        lhsT=outT[c][:, stq * 128 : (stq + 1) * 128],
                        rhs=wo_sb[:, c, :],
                        start=(c == 0),
                        stop=False,
                    )
                nc.tensor.matmul(
                    psy[:], lhsT=ones_row[:], rhs=bop_sb[:], start=False, stop=True,
                )
                nc.vector.tensor_copy(y_acc[:, stq, :], psy[:])
            rcp3 = emit_recip(3)
            for stq in range(8):
                ysb = rpool.tile([128, 512], f32, tag="ysb", name=f"ysb{stq}")
                for hh in range(2):
                    psy = psB.tile([128, 512], f32, tag="psB",
                                   name=f"psyB{stq}_{hh}")
                    nc.tensor.matmul(
                        psy[:],
                        lhsT=outT[3][64 * hh : 64 * hh + 64,
                                     stq * 128 : (stq + 1) * 128],
                        rhs=wo_sb[64 * hh : 64 * hh + 64, 3, :],
                        start=True,
                        stop=True,
                    )
                    nc.vector.scalar_tensor_tensor(
                        ysb[:],
                        psy[:],
                        rcp3[:, stq, 6 + hh : 7 + hh],
                        y_acc[:, stq, :] if hh == 0 else ysb[:],
                        OP.mult,
                        OP.add,
                    )
                nc.sync.dma_start(y[stq * 128 : (stq + 1) * 128, :], ysb[:])

    nc.compile()
    return nc


def _get_nc():
    if "nc" not in _cache:
        _cache["nc"] = _build()
    return _cache["nc"]


def _host_prep(query, key, value, Wq, bq, Wk, bk, Wv, bv, Wo, bo):
    """Shard + transpose + cast inputs for the 8 cores."""
    bf = ml_dtypes.bfloat16
    wqT = np.ascontiguousarray(Wq.T).astype(bf)
    wkT = np.ascontiguousarray(Wk.T).astype(bf)
    wvT = np.ascontiguousarray(Wv.T).astype(bf)
    woT = np.ascontiguousarray(Wo.T).astype(bf)
    bqr = np.ascontiguousarray(bq.reshape(4, 128).T).astype(np.float32)
    bkr = np.ascontiguousarray(bk.reshape(4, 128).T).astype(np.float32)
    bop = (bo + Wo @ bv).astype(np.float32).reshape(1, D).astype(bf)
    ident = np.eye(128, dtype=np.float32)

    in_maps = []
    for c in range(N_CORES):
        b, half = divmod(c, 2)
        xqT = np.ascontiguousarray(
            query[b, half * SQ : (half + 1) * SQ, :].T
        ).astype(bf)
        keyT = np.ascontiguousarray(key[b].T).astype(bf)
        valT = np.ascontiguousarray(value[b].T).astype(bf)
        in_maps.append(
            {
                "xqT": xqT, "keyT": keyT, "valT": valT,
                "wqT": wqT, "wkT": wkT, "wvT": wvT, "woT": woT,
                "bqr": bqr, "bkr": bkr, "bop": bop, "ident": ident,
            }
        )
    return in_maps


def _assemble(results):
    out = np.empty((B, S, D), np.float32)
    for c in range(N_CORES):
        b, half = divmod(c, 2)
        out[b, half * SQ : (half + 1) * SQ, :] = results[c]["y"]
    return out


def _run(in_maps, **spmd_kwargs):
    from concourse.bass_utils import run_bass_kernel_spmd

    nc = _get_nc()
    return run_bass_kernel_spmd(nc, in_maps, list(range(N_CORES)), **spmd_kwargs)


def _reference_fallback(query, key, value, mask, Wq, bq, Wk, bk, Wv, bv, Wo, bo):
    """Exact numpy path, used only if the mask is not all-ones."""
    q = (query @ Wq.T + bq).reshape(B, S, H, HD).transpose(0, 2, 1, 3)
    k = (key @ Wk.T + bk).reshape(B, S, H, HD).transpose(0, 2, 1, 3)
    v = (value @ Wv.T + bv).reshape(B, S, H, HD).transpose(0, 2, 1, 3)
    scores = np.einsum("bhqd,bhkd->bhqk", q, k) / np.sqrt(HD).astype(np.float32)
    scores = np.where(mask[:, None, :, :] == 0, -np.inf, scores)
    scores = scores - scores.max(axis=-1, keepdims=True)
    e = np.exp(scores)
    attn = e / e.sum(axis=-1, keepdims=True)
    x = np.einsum("bhqk,bhkd->bhqd", attn, v)
    x = x.transpose(0, 2, 1, 3).reshape(B, S, D)
    return (x @ Wo.T + bo).astype(np.float32)


def kernel(query, key, value, mask, Wq, bq, Wk, bk, Wv, bv, Wo, bo):
    query = np.asarray(query, np.float32)
    key = np.asarray(key, np.float32)
    value = np.asarray(value, np.float32)
    mask_np = np.asarray(mask)
    args = [
        np.asarray(a, np.float32)
        for a in (Wq, bq, Wk, bk, Wv, bv, Wo, bo)
    ]
    if not np.all(mask_np != 0):
        return _reference_fallback(query, key, value, mask_np, *args)
    in_maps = _host_prep(query, key, value, *args)
    res = _run(in_maps, trace=False)
    return _assemble(res.results)

